# revision 1
# baseline (speedup 1.0000x reference)
"""AFNO2D block-diagonal spectral MLP kernel for 8 Trainium2 NeuronCores.

Math (after simplification of the reference):
  H = W = 128, nb = 8, bs = 96; kept == W so mode truncation is a no-op and
  the imaginary output o2i is discarded by the reference.
  With A1 = w1[0]+w1[1], D1 = w1[0]-w1[1] (same for layer 2):
    o1r = relu(Xk @ (A1/2) + Xn @ (D1/2) + b1[0]/2)
    o1i = relu(Xk @ (D1/2) - Xn @ (D1/2) + b1[1]/2)
    z   = o1r @ (A2/2) + o1i @ (D2/2) + b2[0]/2
    out = x + softshrink(z, 0.01)
  where Xn[b,i,j] = x[b, -i mod H, -j mod W] (pure permutation, done on host
  during sharding). softshrink(z) = relu(z-l) - relu(-z-l)
                                  = relu(z-l) + min(z+l, 0).

Sharding: data-parallel over the 65536 (b,i,j) sites; each core gets 8192
sites. All scale factors folded into the (bf16) weights so every PSUM
readout is a single ScalarE activation (relu, per-partition bias AP) or a
single VectorE tensor_scalar (add-bias -> max/min 0) op.
"""

import numpy as np
import ml_dtypes

import concourse.bass as bass
import concourse.mybir as mybir
from concourse import bacc
from concourse.tile import TileContext
from concourse import bass_utils

BF16 = mybir.dt.bfloat16
F32 = mybir.dt.float32

B, N, C = 4, 16384, 768
H = W = 128
NB, BS = 8, 96
LAMBDA = 0.01
NCORES = 8
SITES = B * N                 # 65536
SITES_PER_CORE = SITES // NCORES   # 8192
TILE = 512                    # sites per site-tile (one PSUM bank)
NTILES = SITES_PER_CORE // TILE    # 16

_cache = {}


def _build():
    nc = bacc.Bacc("TRN2", target_bir_lowering=False)

    xk_d = nc.dram_tensor("xk", [NB, BS, SITES_PER_CORE], BF16, kind="ExternalInput")
    xn_d = nc.dram_tensor("xn", [NB, BS, SITES_PER_CORE], BF16, kind="ExternalInput")
    # weights packed [in_ch=96, block*kind*out_ch]; kinds: A1h, D1h, nD1h, A2h, D2h
    w_d = nc.dram_tensor("w", [BS, NB * 5 * BS], BF16, kind="ExternalInput")
    # biases packed [96, block*4]; kinds: b1r, b1i, bias_a, bias_m
    bias_d = nc.dram_tensor("b", [BS, NB * 4], F32, kind="ExternalInput")
    out_d = nc.dram_tensor("out", [NB, BS, SITES_PER_CORE], BF16, kind="ExternalOutput")

    with TileContext(nc) as tc:
        with (
            tc.tile_pool(name="consts", bufs=1) as consts,
            tc.tile_pool(name="io", bufs=3) as io_pool,
            tc.tile_pool(name="acts", bufs=3) as act_pool,
            tc.tile_pool(name="psum", bufs=2, space="PSUM") as psum_pool,
        ):
            wsb = consts.tile([BS, NB * 5 * BS], BF16)
            nc.sync.dma_start(wsb[:], w_d[:])
            bsb = consts.tile([BS, NB * 4], F32)
            nc.sync.dma_start(bsb[:], bias_d[:])

            def wAP(n, kind):
                return wsb[:, (n * 5 + kind) * BS:(n * 5 + kind + 1) * BS]

            def bAP(n, kind):
                return bsb[:, n * 4 + kind:n * 4 + kind + 1]

            for t in range(NTILES):
                xk_t = io_pool.tile([BS, NB, TILE], BF16, tag="xk")
                xn_t = io_pool.tile([BS, NB, TILE], BF16, tag="xn")
                out_t = io_pool.tile([BS, NB, TILE], BF16, tag="out")
                nc.sync.dma_start(xk_t[:], xk_d[:, :, bass.ts(t, TILE)].rearrange("n c s -> c n s"))
                nc.sync.dma_start(xn_t[:], xn_d[:, :, bass.ts(t, TILE)].rearrange("n c s -> c n s"))

                for n in range(NB):
                    xk_s = xk_t[:, n, :]
                    xn_s = xn_t[:, n, :]

                    p_r = psum_pool.tile([BS, TILE], F32, tag="pr")
                    nc.tensor.matmul(p_r, wAP(n, 0), xk_s, start=True, stop=False)
                    nc.tensor.matmul(p_r, wAP(n, 1), xn_s, start=False, stop=True)

                    p_i = psum_pool.tile([BS, TILE], F32, tag="pi")
                    nc.tensor.matmul(p_i, wAP(n, 1), xk_s, start=True, stop=False)
                    nc.tensor.matmul(p_i, wAP(n, 2), xn_s, start=False, stop=True)

                    o1r = act_pool.tile([BS, TILE], BF16, tag="o1r")
                    nc.scalar.activation(o1r, p_r, mybir.ActivationFunctionType.Relu,
                                         bias=bAP(n, 0), scale=1.0)
                    o1i = act_pool.tile([BS, TILE], BF16, tag="o1i")
                    nc.vector.tensor_scalar(o1i, p_i, bAP(n, 1), 0.0,
                                            mybir.AluOpType.add, mybir.AluOpType.max)

                    p_2 = psum_pool.tile([BS, TILE], F32, tag="p2")
                    nc.tensor.matmul(p_2, wAP(n, 3), o1r, start=True, stop=False)
                    nc.tensor.matmul(p_2, wAP(n, 4), o1i, start=False, stop=True)

                    a_t = act_pool.tile([BS, TILE], BF16, tag="a")
                    nc.scalar.activation(a_t, p_2, mybir.ActivationFunctionType.Relu,
                                         bias=bAP(n, 2), scale=1.0)
                    m_t = act_pool.tile([BS, TILE], BF16, tag="m")
                    nc.vector.tensor_scalar(m_t, p_2, bAP(n, 3), 0.0,
                                            mybir.AluOpType.add, mybir.AluOpType.min)

                    ss_t = act_pool.tile([BS, TILE], BF16, tag="ss")
                    nc.vector.tensor_tensor(ss_t, a_t, m_t, mybir.AluOpType.add)
                    nc.gpsimd.tensor_tensor(out_t[:, n, :], ss_t, xk_s,
                                            mybir.AluOpType.add)

                nc.sync.dma_start(out_d[:, :, bass.ts(t, TILE)].rearrange("n c s -> c n s"),
                                  out_t[:])

    nc.finalize()
    return nc


def _host_prep(x, w1, b1, w2, b2):
    """Shard + permute inputs on host. Returns in_maps for 8 cores."""
    bf = ml_dtypes.bfloat16
    xg = x.reshape(B, H, W, C)
    idx = (-np.arange(H)) % H
    xneg = xg[:, idx][:, :, idx]          # x_neg[b,i,j] = x[b,-i,-j]

    xk_flat = np.ascontiguousarray(
        x.reshape(SITES, C).T.astype(bf)).reshape(NB, BS, SITES)
    xn_flat = np.ascontiguousarray(
        xneg.reshape(SITES, C).T.astype(bf)).reshape(NB, BS, SITES)

    A1h = ((w1[0] + w1[1]) * 0.5)         # [NB, BS, BS] (in, out)
    D1h = ((w1[0] - w1[1]) * 0.5)
    A2h = ((w2[0] + w2[1]) * 0.5)
    D2h = ((w2[0] - w2[1]) * 0.5)
    wpack = np.empty((BS, NB * 5 * BS), dtype=np.float32)
    for n in range(NB):
        for k, mat in enumerate((A1h[n], D1h[n], -D1h[n], A2h[n], D2h[n])):
            wpack[:, (n * 5 + k) * BS:(n * 5 + k + 1) * BS] = mat
    wpack = wpack.astype(bf)

    bpack = np.empty((BS, NB * 4), dtype=np.float32)
    for n in range(NB):
        bpack[:, n * 4 + 0] = b1[0, n] * 0.5
        bpack[:, n * 4 + 1] = b1[1, n] * 0.5
        bpack[:, n * 4 + 2] = b2[0, n] * 0.5 - LAMBDA
        bpack[:, n * 4 + 3] = b2[0, n] * 0.5 + LAMBDA

    in_maps = []
    for c in range(NCORES):
        sl = slice(c * SITES_PER_CORE, (c + 1) * SITES_PER_CORE)
        in_maps.append({
            "xk": np.ascontiguousarray(xk_flat[:, :, sl]),
            "xn": np.ascontiguousarray(xn_flat[:, :, sl]),
            "w": wpack,
            "b": bpack,
        })
    return in_maps


def _assemble(results):
    parts = [r["out"].reshape(C, SITES_PER_CORE) for r in results]
    full = np.concatenate(parts, axis=1)          # [C, SITES]
    return np.ascontiguousarray(full.T).astype(np.float32).reshape(B, N, C)


def _run(x, w1, b1, w2, b2, trace=False):
    if "nc" not in _cache:
        _cache["nc"] = _build()
    nc = _cache["nc"]
    in_maps = _host_prep(x, w1, b1, w2, b2)
    res = bass_utils.run_bass_kernel_spmd(
        nc, in_maps, core_ids=list(range(NCORES)), trace=trace)
    return _assemble(res.results), res


def kernel(x, w1, b1, w2, b2):
    out, _ = _run(x, w1, b1, w2, b2, trace=False)
    return out
